# revision 1
# baseline (speedup 1.0000x reference)
"""Batched KDE kernel for Trainium2 (8 NeuronCores, SPMD).

Problem: out[b, n] = sum_m exp(-||Xq[b,n] - Xf[b,m]||^2 / bw[b])
  with Silverman bandwidth bw[b] from Xf; b=4, n=m=4096, d=32.

Sharding: data-parallel over batch b (4 batches x 2 shards of query rows
= 8 cores). Each core handles n_shard=2048 query rows against the full
m=4096 fit set of its batch.

Device algorithm (per core), raw Bass with manual semaphores:
  psum[n, m] = 2*dot - nmu2 via TWO bf16 K=128 matmuls per 512-col chunk
  (bf16 streams at 1 col/cycle; f32 values are split into bf16 pieces
  x = x1+x2+x3+O(2^-24); Q = 2*Xq^T, f = Xf^T, s = f32(f^2)):
    mmA: lhsT=[q1; q1; q1; -1]   rhs=[f1; f2; f3; s1]
    mmB: lhsT=[q2; q2; q3; -1]   rhs=[f1; f2; f1; s2]
  Sum = q1(f1+f2+f3) + q2(f1+f2) + q3*f1 - s1 - s2 = Q*f - s up to
  O(2^-17)-level dropped cross terms (~2e-4 relative on the exp args).
  ScalarE activation computes exp(psum/bw - nx2/bw) with a fused
  per-partition accumulate (accum_out) -> the sum over m. ACT is the
  bottleneck engine (~1 elem/lane/cycle @ 1.2 GHz).
  nx2 (query norms) is computed on-device from the raw query rows.
Host does sharding/layout/packing plus the 4 scalar bandwidth values
(the global quantile needs a sort, which is pathological on-device).
"""

import os
import numpy as np

B, N, M, D = 4, 4096, 4096, 32
NCORES = 8
SHARDS_PER_BATCH = NCORES // B  # 2
NSHARD = N // SHARDS_PER_BATCH  # 2048
NT = NSHARD // 128  # 16 n-tiles per core
MCHUNK = 512  # matmul free-dim chunk (one psum bank)
ACT_FD = 2048  # activation free dim (4 psum banks)
NG = NT * (M // ACT_FD)  # 32 matmul/exp groups
CPG = ACT_FD // MCHUNK  # psum banks per group = 4

_cached = {}


def _build_program():
    import concourse.bass as bass
    import concourse.mybir as mybir
    from contextlib import ExitStack

    nc = bass.Bass()
    f32 = mybir.dt.float32
    bf16 = mybir.dt.bfloat16

    # stationary operands: only the 96 data rows come from the host; the
    # -1 rows (96:128) are memset on-device
    la = nc.declare_dram_parameter("la", [96, NSHARD], bf16, isOutput=False)
    lb = nc.declare_dram_parameter("lb", [96, NSHARD], bf16, isOutput=False)
    ra = nc.declare_dram_parameter("ra", [128, M], bf16, isOutput=False)
    rb = nc.declare_dram_parameter("rb", [128, M], bf16, isOutput=False)
    XQN_W = NT * D + 1 + NT
    xqn = nc.declare_dram_parameter("xqn", [128, XQN_W], f32, isOutput=False)
    res = nc.declare_dram_parameter("res", [128, NT], f32, isOutput=True)

    NLC = 2  # 1024-col chunks of la/lb
    NRC = 4  # 1024-col chunks of ra/rb
    LW_ = NSHARD // NLC
    RW = M // NRC

    with ExitStack() as ctx:
        la_sb = ctx.enter_context(nc.sbuf_tensor([128, NSHARD], bf16))
        lb_sb = ctx.enter_context(nc.sbuf_tensor([128, NSHARD], bf16))
        ra_sb = ctx.enter_context(nc.sbuf_tensor([128, M], bf16))
        rb_sb = ctx.enter_context(nc.sbuf_tensor([128, M], bf16))
        xqn_sb = ctx.enter_context(nc.sbuf_tensor([128, XQN_W], f32))
        sq = ctx.enter_context(nc.sbuf_tensor([128, NT * D], f32))
        nx2r = ctx.enter_context(nc.sbuf_tensor([128, NT], f32))
        bias_all = ctx.enter_context(nc.sbuf_tensor([128, NT], f32))
        # slot 0..NG-1 = regular groups; slot NG = the split-off first
        # half-group (banks 0-1 of group 0, summed into res col 0 at the end)
        acc = ctx.enter_context(nc.sbuf_tensor([128, NG + 1], f32))
        res_sb = ctx.enter_context(nc.sbuf_tensor([128, NT], f32))
        warmT = ctx.enter_context(nc.sbuf_tensor([1, 1], f32))
        escr0 = ctx.enter_context(nc.sbuf_tensor([128, ACT_FD], bf16))
        escr1 = ctx.enter_context(nc.sbuf_tensor([128, ACT_FD], bf16))
        escr = [escr0, escr1]
        ps0 = ctx.enter_context(nc.psum_tensor("ps0", [128, ACT_FD], f32))
        ps1 = ctx.enter_context(nc.psum_tensor("ps1", [128, ACT_FD], f32))
        ps = [ps0, ps1]

        sem_xqn = ctx.enter_context(nc.semaphore("sem_xqn"))
        s_la = [ctx.enter_context(nc.semaphore(f"s_la{i}")) for i in range(NLC)]
        s_lb = [ctx.enter_context(nc.semaphore(f"s_lb{i}")) for i in range(NLC)]
        s_ra = [ctx.enter_context(nc.semaphore(f"s_ra{i}")) for i in range(NRC)]
        s_rb = [ctx.enter_context(nc.semaphore(f"s_rb{i}")) for i in range(NRC)]
        sem_out = ctx.enter_context(nc.semaphore("sem_out"))
        s_warm = ctx.enter_context(nc.semaphore("s_warm"))
        s_dve = ctx.enter_context(nc.semaphore("s_dve"))
        s_act = ctx.enter_context(nc.semaphore("s_act"))
        s_pe = ctx.enter_context(nc.semaphore("s_pe"))
        block = ctx.enter_context(nc.Block())

        scale_pos = xqn_sb[:, NT * D : NT * D + 1]  # 1/bw
        neg_invbw = xqn_sb[:, NT * D + 1 : NT * D + 1 + NT]  # -1/bw x NT

        @block.sync
        def _(sync):
            # critical first-half chunks; the rest is deferred until these
            # have landed so they don't compete for HBM bandwidth (the
            # second m-half isn't consumed until halfway through the run)
            sync.dma_start(xqn_sb[:], xqn[:]).then_inc(sem_xqn, 16)
            sync.dma_start(la_sb[0:96, 0:LW_], la[:, 0:LW_]).then_inc(s_la[0], 16)
            sync.dma_start(lb_sb[0:96, 0:LW_], lb[:, 0:LW_]).then_inc(s_lb[0], 16)
            sync.dma_start(ra_sb[:, 0:RW], ra[:, 0:RW]).then_inc(s_ra[0], 16)
            sync.dma_start(rb_sb[:, 0:RW], rb[:, 0:RW]).then_inc(s_rb[0], 16)
            sync.dma_start(
                ra_sb[:, RW : 2 * RW], ra[:, RW : 2 * RW]
            ).then_inc(s_ra[1], 16)
            sync.dma_start(
                rb_sb[:, RW : 2 * RW], rb[:, RW : 2 * RW]
            ).then_inc(s_rb[1], 16)
            sync.wait_ge(s_rb[1], 16)
            sync.dma_start(
                la_sb[0:96, LW_ : 2 * LW_], la[:, LW_ : 2 * LW_]
            ).then_inc(s_la[1], 16)
            sync.dma_start(
                lb_sb[0:96, LW_ : 2 * LW_], lb[:, LW_ : 2 * LW_]
            ).then_inc(s_lb[1], 16)
            for i in range(2, NRC):
                sync.dma_start(
                    ra_sb[:, i * RW : (i + 1) * RW], ra[:, i * RW : (i + 1) * RW]
                ).then_inc(s_ra[i], 16)
                sync.dma_start(
                    rb_sb[:, i * RW : (i + 1) * RW], rb[:, i * RW : (i + 1) * RW]
                ).then_inc(s_rb[i], 16)
            sync.wait_ge(s_dve, 8)
            sync.dma_start(res[:], res_sb[:]).then_inc(sem_out, 16)
            sync.wait_ge(sem_out, 16)

        @block.vector
        def _(vector):
            # scratch for the PE warmup matmuls
            nc.vector.memset(escr0[:, 0:MCHUNK], 0.0).then_inc(s_dve, 1)
            # -1 rows of the stationary operands
            nc.vector.memset(la_sb[96:128, :], -1.0).then_inc(s_dve, 1)
            nc.vector.memset(lb_sb[96:128, :], -1.0).then_inc(s_dve, 1)
            vector.wait_ge(sem_xqn, 16)
            nc.vector.tensor_tensor(
                sq[:],
                xqn_sb[:, : NT * D],
                xqn_sb[:, : NT * D],
                op=mybir.AluOpType.mult,
            ).then_inc(s_dve, 1)
            vector.wait_ge(s_dve, 4)
            nc.vector.tensor_reduce(
                nx2r[:],
                sq[:].rearrange("p (t d) -> p t d", d=D),
                axis=mybir.AxisListType.X,
                op=mybir.AluOpType.add,
            ).then_inc(s_dve, 1)
            vector.wait_ge(s_dve, 5)
            nc.vector.tensor_tensor(
                bias_all[:], nx2r[:], neg_invbw, op=mybir.AluOpType.mult
            ).then_inc(s_dve, 1)
            vector.wait_ge(s_act, NG + 1)
            nc.vector.tensor_reduce(
                res_sb[:],
                acc[:, :NG].rearrange("p (t h) -> p t h", h=M // ACT_FD),
                axis=mybir.AxisListType.X,
                op=mybir.AluOpType.add,
            ).then_inc(s_dve, 1)
            vector.wait_ge(s_dve, 7)
            nc.vector.tensor_tensor(
                res_sb[:, 0:1],
                res_sb[:, 0:1],
                acc[:, NG : NG + 1],
                op=mybir.AluOpType.add,
            ).then_inc(s_dve, 1)

        @block.scalar
        def _(scalar):
            # trigger the exp table-set DMA during the kernel head so the
            # first real exp doesn't pay the ~1.3us ACT_TABLE_LOAD
            nc.scalar.memzero(warmT[:]).then_inc(s_warm, 1)
            scalar.wait_ge(s_warm, 1)
            nc.scalar.activation(
                warmT[:], warmT[:], mybir.ActivationFunctionType.Exp
            )
            scalar.wait_ge(sem_xqn, 16)
            scalar.wait_ge(s_dve, 6)
            # group 0 is split in two so the first exp can start as soon as
            # the first 1024 columns of psum are ready
            scalar.wait_ge(s_pe, 1)
            nc.scalar.activation(
                escr[0][:, 0 : ACT_FD // 2],
                ps[0][:, 0 : ACT_FD // 2],
                mybir.ActivationFunctionType.Exp,
                bias=bias_all[:, 0:1],
                scale=scale_pos,
                accum_out=acc[:, NG : NG + 1],
            ).then_inc(s_act, 1)
            scalar.wait_ge(s_pe, 2)
            nc.scalar.activation(
                escr[0][:, ACT_FD // 2 :],
                ps[0][:, ACT_FD // 2 :],
                mybir.ActivationFunctionType.Exp,
                bias=bias_all[:, 0:1],
                scale=scale_pos,
                accum_out=acc[:, 0:1],
            ).then_inc(s_act, 1)
            for g in range(1, NG):
                t = g % NT
                scalar.wait_ge(s_pe, g + 2)
                slot = t * (M // ACT_FD) + (g // NT)
                nc.scalar.activation(
                    escr[g % 2][:],
                    ps[g % 2][:],
                    mybir.ActivationFunctionType.Exp,
                    bias=bias_all[:, t : t + 1],
                    scale=scale_pos,
                    accum_out=acc[:, slot : slot + 1],
                ).then_inc(s_act, 1)

        @block.tensor
        def _(tensor):
            # warm the PE clock (HAM) during the head with dummy matmuls on
            # the memset -1 rows, so the first real groups run at 2.4 GHz
            tensor.wait_ge(s_dve, 1)  # warmup scratch memset
            for _w in range(16):
                nc.tensor.matmul(
                    ps0[:, 0:MCHUNK],
                    escr0[:, 0:128],
                    escr0[:, 0:MCHUNK],
                    start=True,
                    stop=True,
                )
            tensor.wait_ge(s_dve, 3)  # la/lb -1 rows memset
            g = 0
            # h-outer: the second m-half (ra/rb chunks 2-3) is first
            # touched halfway through the kernel, so its DMA is deferred
            for h in range(M // ACT_FD):
                for t in range(NT):
                    if t % (NT // NLC) == 0:
                        c = t // (NT // NLC)
                        tensor.wait_ge(s_la[c], 16)
                        tensor.wait_ge(s_lb[c], 16)
                    lsl = slice(t * 128, (t + 1) * 128)
                    if g >= 2:
                        tensor.wait_ge(s_act, g)
                    pg = ps[g % 2]
                    if g == 0:
                        # split: banks 0-1 then banks 2-3, an inc after
                        # each half so the first exp starts early
                        for half in range(2):
                            for j in range(2 * half, 2 * half + 2):
                                if j % 2 == 0:
                                    tensor.wait_ge(s_ra[j // 2], 16)
                                m0 = j * MCHUNK
                                nc.tensor.matmul(
                                    pg[:, j * MCHUNK : (j + 1) * MCHUNK],
                                    la_sb[:, lsl],
                                    ra_sb[:, m0 : m0 + MCHUNK],
                                    start=True,
                                    stop=False,
                                )
                            for j in range(2 * half, 2 * half + 2):
                                if j % 2 == 0:
                                    tensor.wait_ge(s_rb[j // 2], 16)
                                m0 = j * MCHUNK
                                mm = nc.tensor.matmul(
                                    pg[:, j * MCHUNK : (j + 1) * MCHUNK],
                                    lb_sb[:, lsl],
                                    rb_sb[:, m0 : m0 + MCHUNK],
                                    start=False,
                                    stop=True,
                                )
                                if j % 2 == 1:
                                    mm.then_inc(s_pe, 1)
                        g += 1
                        continue
                    for j in range(CPG):
                        if t == 0 and j % 2 == 0:
                            tensor.wait_ge(s_ra[2 * h + j // 2], 16)
                        m0 = h * ACT_FD + j * MCHUNK
                        nc.tensor.matmul(
                            pg[:, j * MCHUNK : (j + 1) * MCHUNK],
                            la_sb[:, lsl],
                            ra_sb[:, m0 : m0 + MCHUNK],
                            start=True,
                            stop=False,
                        )
                    for j in range(CPG):
                        if t == 0 and j % 2 == 0:
                            tensor.wait_ge(s_rb[2 * h + j // 2], 16)
                        m0 = h * ACT_FD + j * MCHUNK
                        mm = nc.tensor.matmul(
                            pg[:, j * MCHUNK : (j + 1) * MCHUNK],
                            lb_sb[:, lsl],
                            rb_sb[:, m0 : m0 + MCHUNK],
                            start=False,
                            stop=True,
                        )
                        if j == CPG - 1:
                            mm.then_inc(s_pe, 1)
                    g += 1

    return nc


def _bf16_split3(x):
    import ml_dtypes

    bf = ml_dtypes.bfloat16
    x = x.astype(np.float32)
    p1 = x.astype(bf)
    rem = x - p1.astype(np.float32)
    p2 = rem.astype(bf)
    rem2 = rem - p2.astype(np.float32)
    p3 = rem2.astype(bf)
    return p1, p2, p3


def _bandwidth_np(X_fit):
    # mirror of reference._bandwidth (Silverman-style)
    b, n, d = X_fit.shape
    flat = np.asarray(X_fit, dtype=np.float64).reshape(-1)
    q = np.quantile(flat, 0.75) - np.quantile(flat, 0.25)
    std = np.std(np.asarray(X_fit, dtype=np.float64).reshape(b, -1), axis=1, ddof=1)
    return (0.9 * np.minimum(std, q / 1.34) / (n**0.2)).astype(np.float32)


def _host_prep(X_query, X_fit):
    X_query = np.asarray(X_query, dtype=np.float32)
    X_fit = np.asarray(X_fit, dtype=np.float32)
    bw = _bandwidth_np(X_fit)  # [B]

    in_maps = []
    for c in range(NCORES):
        b = c // SHARDS_PER_BATCH
        s = c % SHARDS_PER_BATCH
        XQ = X_query[b, s * NSHARD : (s + 1) * NSHARD]  # [2048, 32]
        XF = X_fit[b]  # [4096, 32]

        # permuted queries: tile t / partition p handles query row p*NT + t
        XQp = XQ.reshape(128, NT, D).transpose(1, 0, 2).reshape(NSHARD, D)
        Q = np.ascontiguousarray((2.0 * XQp.T).astype(np.float32))  # [32, 2048]
        q1, q2, q3 = _bf16_split3(Q)
        FT = np.ascontiguousarray(XF.T.astype(np.float32))  # [32, 4096]
        f1, f2, f3 = _bf16_split3(FT)
        sqr = FT * FT  # f32-rounded squares, matches reference nmu2 terms
        s1, s2, _s3 = _bf16_split3(sqr)

        la_np = np.concatenate([q1, q1, q1], axis=0)  # [96, 2048]
        lb_np = np.concatenate([q2, q2, q3], axis=0)
        ra_np = np.concatenate([f1, f2, f3, s1], axis=0)  # [128, 4096]
        rb_np = np.concatenate([f1, f2, f1, s2], axis=0)

        inv_bw = np.float32(1.0) / bw[b]
        xqn = np.empty((128, NT * D + 1 + NT), dtype=np.float32)
        xqn[:, : NT * D] = XQ.reshape(128, NT * D)
        xqn[:, NT * D] = inv_bw
        xqn[:, NT * D + 1 :] = -inv_bw

        in_maps.append(
            {"la": la_np, "lb": lb_np, "ra": ra_np, "rb": rb_np, "xqn": xqn}
        )
    return in_maps


def _gather(results):
    out = np.empty((B, N), dtype=np.float32)
    for c in range(NCORES):
        b = c // SHARDS_PER_BATCH
        s = c % SHARDS_PER_BATCH
        res = np.asarray(results[c]["res"], dtype=np.float32)  # [128, 16]
        out[b, s * NSHARD : (s + 1) * NSHARD] = res.reshape(NSHARD)
    return out


def kernel(X_query, X_fit):
    from concourse.bass_utils import run_bass_kernel_spmd

    if "nc" not in _cached:
        _cached["nc"] = _build_program()
    nc = _cached["nc"]
    in_maps = _host_prep(X_query, X_fit)
    out = run_bass_kernel_spmd(nc, in_maps, list(range(NCORES)))
    return _gather(out.results)



# revision 16
# speedup vs baseline: 1.0199x; 1.0199x over previous
"""Batched KDE kernel for Trainium2 (8 NeuronCores, SPMD).

Problem: out[b, n] = sum_m exp(-||Xq[b,n] - Xf[b,m]||^2 / bw[b])
  with Silverman bandwidth bw[b] from Xf; b=4, n=m=4096, d=32.

Sharding: data-parallel over batch b (4 batches x 2 shards of query rows
= 8 cores). Each core handles n_shard=2048 query rows against the full
m=4096 fit set of its batch.

Device algorithm (per core), raw Bass with manual semaphores:
  psum[n, m] = 2*dot - nmu2 via TWO bf16 K=128 matmuls per 512-col chunk
  (bf16 streams at 1 col/cycle; f32 values are split into bf16 pieces
  x = x1+x2+x3+O(2^-24); Q = 2*Xq^T, f = Xf^T, s = f32(f^2)):
    mmA: lhsT=[q1; q1; q1; -1]   rhs=[f1; f2; f3; s1]
    mmB: lhsT=[q2; q2; q3; -1]   rhs=[f1; f2; f1; s2]
  ScalarE activation computes exp(psum/bw - nx2/bw) with a fused
  per-partition accumulate (accum_out) -> the sum over m. ACT is the
  bottleneck (~2.05us per 2048-col group); the schedule keeps its exp
  stream dense and starts it early:
    - bias (-nx2/bw) and scale (1/bw) are host-computed, riding in the
      first scalar-queue DMA (f32 bytes bitcast into the bf16 blob)
    - inputs live in ONE dram blob ordered by first-use, split across
      BOTH HWDGE queues (sync + scalar) for ~2x head bandwidth
    - -1 rows are baked into the blob (no memsets ahead of the PE)
    - PE warmup matmuls on garbage SBUF start immediately so the HAM
      clock gate is released before the first real group
    - exp output is written back IN PLACE to the psum region it reads
      (discarded data; ScalarE->PSUM is the faster port)
    - the final acc->res reduction is split in two so only half of it
      sits after the last exp group
  NOTE: engines run in relaxed ordering mode — any same-engine RAW
  (e.g. vector reduce then add on the same column) needs an explicit
  semaphore between producer and consumer.
Host does sharding/layout/packing plus the 4 scalar bandwidth values and
query norms (global quantile needs a sort; both are O(input) prep).
"""

import numpy as np

B, N, M, D = 4, 4096, 4096, 32
NCORES = 8
SHARDS_PER_BATCH = NCORES // B  # 2
NSHARD = N // SHARDS_PER_BATCH  # 2048
NT = NSHARD // 128  # 16 n-tiles per core
MCHUNK = 512  # matmul free-dim chunk (one psum bank)
ACT_FD = 2048  # activation free dim (4 psum banks)
NG = NT * (M // ACT_FD)  # 32 matmul/exp groups

# blob column offsets (bf16 cols, 64-aligned), ordered by first use and
# grouped into contiguous per-queue transfer ranges
OFF_A = 0  # la tile0 (128)
OFF_B = 128  # lb tile0 (128)
OFF_CA = 256  # ra m[0:512)
OFF_DA = 768  # rb m[0:512)
OFF_M = 1280  # meta: f32 [128,17] bitcast -> 34 bf16 cols (padded to 64)
OFF_CB = 1344  # ra m[512:1024)
OFF_DB = 1856  # rb m[512:1024)
OFF_G = 2368  # ra m[1024:2048)
OFF_H = 3392  # rb m[1024:2048)
OFF_E1 = 4416  # la t1-4 (512)
OFF_F1 = 4928  # lb t1-4 (512)
OFF_E2 = 5440  # la t5-15 (1408)
OFF_F2 = 6848  # lb t5-15 (1408)
OFF_I = 8256  # ra m[2048:4096) (2048)
OFF_J = 10304  # rb m[2048:4096) (2048)
BLOB_W = 12352

_cached = {}


def _la_off(t):
    if t == 0:
        return OFF_A
    if t <= 4:
        return OFF_E1 + (t - 1) * 128
    return OFF_E2 + (t - 5) * 128


def _lb_off(t):
    if t == 0:
        return OFF_B
    if t <= 4:
        return OFF_F1 + (t - 1) * 128
    return OFF_F2 + (t - 5) * 128


def _ra_off(c):  # c = m-col / 512, 0..7
    if c == 0:
        return OFF_CA
    if c == 1:
        return OFF_CB
    if c <= 3:
        return OFF_G + (c - 2) * 512
    return OFF_I + (c - 4) * 512


def _rb_off(c):
    if c == 0:
        return OFF_DA
    if c == 1:
        return OFF_DB
    if c <= 3:
        return OFF_H + (c - 2) * 512
    return OFF_J + (c - 4) * 512


def _build_program():
    import concourse.bass as bass
    import concourse.mybir as mybir
    from contextlib import ExitStack

    nc = bass.Bass()
    f32 = mybir.dt.float32
    bf16 = mybir.dt.bfloat16

    blob = nc.declare_dram_parameter("blob", [128, BLOB_W], bf16, isOutput=False)
    res = nc.declare_dram_parameter("res", [128, NT], f32, isOutput=True)

    with ExitStack() as ctx:
        msb = ctx.enter_context(nc.sbuf_tensor([128, BLOB_W], bf16))
        # slot 2t+h per group; slot NG = split-off first half of group 0
        acc = ctx.enter_context(nc.sbuf_tensor([128, NG + 1], f32))
        res_sb = ctx.enter_context(nc.sbuf_tensor([128, NT], f32))
        warmT = ctx.enter_context(nc.sbuf_tensor([128, 1], f32))
        wscr = ctx.enter_context(nc.sbuf_tensor([128, 640], bf16))
        ps0 = ctx.enter_context(nc.psum_tensor("ps0", [128, ACT_FD], f32))
        ps1 = ctx.enter_context(nc.psum_tensor("ps1", [128, ACT_FD], f32))
        ps = [ps0, ps1]

        s_h1 = ctx.enter_context(nc.semaphore("s_h1"))
        s_m = ctx.enter_context(nc.semaphore("s_m"))
        s_g = ctx.enter_context(nc.semaphore("s_g"))
        s_h2 = ctx.enter_context(nc.semaphore("s_h2"))
        s_ef1 = ctx.enter_context(nc.semaphore("s_ef1"))
        s_ef2 = ctx.enter_context(nc.semaphore("s_ef2"))
        s_i = ctx.enter_context(nc.semaphore("s_i"))
        s_j = ctx.enter_context(nc.semaphore("s_j"))
        s_pe = ctx.enter_context(nc.semaphore("s_pe"))
        s_act = ctx.enter_context(nc.semaphore("s_act"))
        s_v1 = ctx.enter_context(nc.semaphore("s_v1"))
        s_v2 = ctx.enter_context(nc.semaphore("s_v2"))
        sem_out = ctx.enter_context(nc.semaphore("sem_out"))
        block = ctx.enter_context(nc.Block())

        meta32 = msb[:, OFF_M : OFF_M + 34].bitcast(f32)  # [128, 17]
        scale_pos = meta32[:, 16:17]  # 1/bw
        # meta32[:, t] = -nx2/bw for tile t

        @block.sync
        def _(sync):
            sync.dma_start(msb[:, 0:OFF_M], blob[:, 0:OFF_M]).then_inc(s_h1, 16)
            sync.dma_start(msb[:, OFF_G:OFF_H], blob[:, OFF_G:OFF_H]).then_inc(
                s_g, 16
            )
            sync.dma_start(
                msb[:, OFF_E1:OFF_E2], blob[:, OFF_E1:OFF_E2]
            ).then_inc(s_ef1, 16)
            sync.dma_start(msb[:, OFF_I:OFF_J], blob[:, OFF_I:OFF_J]).then_inc(
                s_i, 16
            )
            sync.dma_start(msb[:, OFF_J:BLOB_W], blob[:, OFF_J:BLOB_W]).then_inc(
                s_j, 16
            )
            sync.wait_ge(s_v1, 2)
            sync.dma_start(res[:, 0:8], res_sb[:, 0:8]).then_inc(sem_out, 16)
            sync.wait_ge(s_v2, 1)
            sync.dma_start(res[:, 8:16], res_sb[:, 8:16]).then_inc(sem_out, 16)
            sync.wait_ge(sem_out, 32)

        @block.vector
        def _(vector):
            # split final reduction: tiles 0-7 as soon as their h=1 groups
            # are done, tiles 8-15 after the last group
            vector.wait_ge(s_act, 2 + 23)
            # relaxed ordering: same-engine RAW needs an explicit semaphore
            # between the reduce (writes res_sb[:,0]) and the add (reads it)
            nc.vector.tensor_reduce(
                res_sb[:, 0:8],
                acc[:, 0:16].rearrange("p (t h) -> p t h", h=2),
                axis=mybir.AxisListType.X,
                op=mybir.AluOpType.add,
            ).then_inc(s_v1, 1)
            vector.wait_ge(s_v1, 1)
            nc.vector.tensor_tensor(
                res_sb[:, 0:1],
                res_sb[:, 0:1],
                acc[:, NG : NG + 1],
                op=mybir.AluOpType.add,
            ).then_inc(s_v1, 1)  # s_v1 == 2: phase-1 output ready
            vector.wait_ge(s_act, 2 + 31)
            nc.vector.tensor_reduce(
                res_sb[:, 8:16],
                acc[:, 16:32].rearrange("p (t h) -> p t h", h=2),
                axis=mybir.AxisListType.X,
                op=mybir.AluOpType.add,
            ).then_inc(s_v2, 1)

        @block.scalar
        def _(scalar):
            # second HWDGE queue: scalar-issued DMAs run in parallel with
            # the sync queue, roughly doubling head bandwidth
            scalar.dma_start(msb[:, OFF_M:OFF_G], blob[:, OFF_M:OFF_G]).then_inc(
                s_m, 16
            )
            scalar.dma_start(msb[:, OFF_H:OFF_E1], blob[:, OFF_H:OFF_E1]).then_inc(
                s_h2, 16
            )
            scalar.dma_start(
                msb[:, OFF_E2:OFF_I], blob[:, OFF_E2:OFF_I]
            ).then_inc(s_ef2, 16)
            # fire the exp table-set load; operands are garbage (meta not
            # yet DMA'd) but the output is discarded
            nc.scalar.activation(
                warmT[:],
                warmT[:],
                mybir.ActivationFunctionType.Exp,
                bias=meta32[:, 0:1],
            )
            scalar.wait_ge(s_m, 16)
            # group 0 split in two 1024-col halves for an earlier start;
            # exp output overwrites the psum region it just read
            scalar.wait_ge(s_pe, 1)
            nc.scalar.activation(
                ps0[:, 0:1024],
                ps0[:, 0:1024],
                mybir.ActivationFunctionType.Exp,
                bias=meta32[:, 0:1],
                scale=scale_pos,
                accum_out=acc[:, NG : NG + 1],
            ).then_inc(s_act, 1)
            scalar.wait_ge(s_pe, 2)
            nc.scalar.activation(
                ps0[:, 1024:2048],
                ps0[:, 1024:2048],
                mybir.ActivationFunctionType.Exp,
                bias=meta32[:, 0:1],
                scale=scale_pos,
                accum_out=acc[:, 0:1],
            ).then_inc(s_act, 1)
            for g in range(1, NG):
                t = g % NT
                slot = 2 * t + (g // NT)
                scalar.wait_ge(s_pe, g + 2)
                nc.scalar.activation(
                    ps[g % 2][:],
                    ps[g % 2][:],
                    mybir.ActivationFunctionType.Exp,
                    bias=meta32[:, t : t + 1],
                    scale=scale_pos,
                    accum_out=acc[:, slot : slot + 1],
                ).then_inc(s_act, 1)

        @block.tensor
        def _(tensor):
            # warm the PE clock (HAM) with dummy matmuls on garbage SBUF so
            # group 0 runs at 2.4 GHz; ps0 is overwritten by group 0
            for _w in range(8):
                nc.tensor.matmul(
                    ps0[:, 0:MCHUNK],
                    wscr[:, 0:128],
                    wscr[:, 128:640],
                    start=True,
                    stop=True,
                )
            for g in range(NG):
                t = g % NT
                h = g // NT
                pg = ps[g % 2]
                la = msb[:, _la_off(t) : _la_off(t) + 128]
                lb = msb[:, _lb_off(t) : _lb_off(t) + 128]
                if g == 0:
                    # chunk 0 from sync queue, chunk 1 from scalar queue,
                    # chunks 2 (ra) / 3 (rb) from their own transfers
                    tensor.wait_ge(s_h1, 16)
                    nc.tensor.matmul(
                        pg[:, 0:MCHUNK],
                        la,
                        msb[:, _ra_off(0) : _ra_off(0) + MCHUNK],
                        start=True,
                        stop=False,
                    )
                    nc.tensor.matmul(
                        pg[:, 0:MCHUNK],
                        lb,
                        msb[:, _rb_off(0) : _rb_off(0) + MCHUNK],
                        start=False,
                        stop=True,
                    )
                    tensor.wait_ge(s_m, 16)
                    nc.tensor.matmul(
                        pg[:, MCHUNK : 2 * MCHUNK],
                        la,
                        msb[:, _ra_off(1) : _ra_off(1) + MCHUNK],
                        start=True,
                        stop=False,
                    )
                    nc.tensor.matmul(
                        pg[:, MCHUNK : 2 * MCHUNK],
                        lb,
                        msb[:, _rb_off(1) : _rb_off(1) + MCHUNK],
                        start=False,
                        stop=True,
                    ).then_inc(s_pe, 1)
                    tensor.wait_ge(s_g, 16)
                    for c in (2, 3):
                        nc.tensor.matmul(
                            pg[:, c * MCHUNK : (c + 1) * MCHUNK],
                            la,
                            msb[:, _ra_off(c) : _ra_off(c) + MCHUNK],
                            start=True,
                            stop=False,
                        )
                    tensor.wait_ge(s_h2, 16)
                    for c in (2, 3):
                        mm = nc.tensor.matmul(
                            pg[:, c * MCHUNK : (c + 1) * MCHUNK],
                            lb,
                            msb[:, _rb_off(c) : _rb_off(c) + MCHUNK],
                            start=False,
                            stop=True,
                        )
                        if c == 3:
                            mm.then_inc(s_pe, 1)
                    continue
                if g == 1:
                    tensor.wait_ge(s_ef1, 16)
                if t == 5 and h == 0:
                    tensor.wait_ge(s_ef2, 16)
                if g == 16:
                    tensor.wait_ge(s_i, 16)
                if g >= 2:
                    tensor.wait_ge(s_act, g)
                for j in range(4):
                    c = 4 * h + j
                    nc.tensor.matmul(
                        pg[:, j * MCHUNK : (j + 1) * MCHUNK],
                        la,
                        msb[:, _ra_off(c) : _ra_off(c) + MCHUNK],
                        start=True,
                        stop=False,
                    )
                if g == 16:
                    tensor.wait_ge(s_j, 16)
                for j in range(4):
                    c = 4 * h + j
                    mm = nc.tensor.matmul(
                        pg[:, j * MCHUNK : (j + 1) * MCHUNK],
                        lb,
                        msb[:, _rb_off(c) : _rb_off(c) + MCHUNK],
                        start=False,
                        stop=True,
                    )
                    if j == 3:
                        mm.then_inc(s_pe, 1)

    return nc


def _bf16_split3(x):
    import ml_dtypes

    bf = ml_dtypes.bfloat16
    x = x.astype(np.float32)
    p1 = x.astype(bf)
    rem = x - p1.astype(np.float32)
    p2 = rem.astype(bf)
    rem2 = rem - p2.astype(np.float32)
    p3 = rem2.astype(bf)
    return p1, p2, p3


def _bandwidth_np(X_fit):
    # mirror of reference._bandwidth (Silverman-style)
    b, n, d = X_fit.shape
    flat = np.asarray(X_fit, dtype=np.float64).reshape(-1)
    q = np.quantile(flat, 0.75) - np.quantile(flat, 0.25)
    std = np.std(np.asarray(X_fit, dtype=np.float64).reshape(b, -1), axis=1, ddof=1)
    return (0.9 * np.minimum(std, q / 1.34) / (n**0.2)).astype(np.float32)


def _host_prep(X_query, X_fit):
    import ml_dtypes

    bf = ml_dtypes.bfloat16
    X_query = np.asarray(X_query, dtype=np.float32)
    X_fit = np.asarray(X_fit, dtype=np.float32)
    bw = _bandwidth_np(X_fit)  # [B]

    in_maps = []
    for c in range(NCORES):
        b = c // SHARDS_PER_BATCH
        s = c % SHARDS_PER_BATCH
        XQ = X_query[b, s * NSHARD : (s + 1) * NSHARD]  # [2048, 32]
        XF = X_fit[b]  # [4096, 32]

        # permuted queries: tile t / partition p handles query row p*NT + t
        XQp = XQ.reshape(128, NT, D).transpose(1, 0, 2).reshape(NSHARD, D)
        Q = np.ascontiguousarray((2.0 * XQp.T).astype(np.float32))  # [32, 2048]
        q1, q2, q3 = _bf16_split3(Q)
        FT = np.ascontiguousarray(XF.T.astype(np.float32))  # [32, 4096]
        f1, f2, f3 = _bf16_split3(FT)
        sqr = FT * FT  # f32-rounded squares, matches reference nmu2 terms
        s1, s2, _s3 = _bf16_split3(sqr)

        neg1 = np.full((32, NSHARD), -1.0, dtype=bf)
        la = np.concatenate([q1, q1, q1, neg1], axis=0)  # [128, 2048]
        lb = np.concatenate([q2, q2, q3, neg1], axis=0)
        ra = np.concatenate([f1, f2, f3, s1], axis=0)  # [128, 4096]
        rb = np.concatenate([f1, f2, f1, s2], axis=0)

        inv_bw = np.float32(1.0) / bw[b]
        nx2 = (XQ.reshape(128, NT, D).astype(np.float64) ** 2).sum(-1)
        meta = np.empty((128, 17), dtype=np.float32)
        meta[:, 0:16] = (-nx2 * np.float64(inv_bw)).astype(np.float32)
        meta[:, 16] = inv_bw

        blob = np.zeros((128, BLOB_W), dtype=bf)
        blob[:, OFF_A : OFF_A + 128] = la[:, 0:128]
        blob[:, OFF_B : OFF_B + 128] = lb[:, 0:128]
        blob[:, OFF_CA : OFF_CA + 512] = ra[:, 0:512]
        blob[:, OFF_DA : OFF_DA + 512] = rb[:, 0:512]
        blob[:, OFF_M : OFF_M + 34] = meta.view(np.uint16).view(bf)  # raw bytes
        blob[:, OFF_CB : OFF_CB + 512] = ra[:, 512:1024]
        blob[:, OFF_DB : OFF_DB + 512] = rb[:, 512:1024]
        blob[:, OFF_G : OFF_G + 1024] = ra[:, 1024:2048]
        blob[:, OFF_H : OFF_H + 1024] = rb[:, 1024:2048]
        blob[:, OFF_E1 : OFF_E1 + 512] = la[:, 128:640]
        blob[:, OFF_F1 : OFF_F1 + 512] = lb[:, 128:640]
        blob[:, OFF_E2 : OFF_E2 + 1408] = la[:, 640:2048]
        blob[:, OFF_F2 : OFF_F2 + 1408] = lb[:, 640:2048]
        blob[:, OFF_I : OFF_I + 2048] = ra[:, 2048:4096]
        blob[:, OFF_J : OFF_J + 2048] = rb[:, 2048:4096]

        in_maps.append({"blob": blob})
    return in_maps


def _gather(results):
    out = np.empty((B, N), dtype=np.float32)
    for c in range(NCORES):
        b = c // SHARDS_PER_BATCH
        s = c % SHARDS_PER_BATCH
        res = np.asarray(results[c]["res"], dtype=np.float32)  # [128, 16]
        out[b, s * NSHARD : (s + 1) * NSHARD] = res.reshape(NSHARD)
    return out


def kernel(X_query, X_fit):
    from concourse.bass_utils import run_bass_kernel_spmd

    if "nc" not in _cached:
        _cached["nc"] = _build_program()
    nc = _cached["nc"]
    in_maps = _host_prep(X_query, X_fit)
    out = run_bass_kernel_spmd(nc, in_maps, list(range(NCORES)))
    return _gather(out.results)
